# revision 5
# baseline (speedup 1.0000x reference)
"""KNN retrieval kernel (DGM graph construction) for 8 Trainium2 NeuronCores.

Problem: for x [16384, 64] fp32, find each row's 20 nearest neighbors under
squared euclidean distance, return (x[None], edges [2, N*K] int32,
logprobs [1, N, K] fp32) exactly as the reference jax module does.

Strategy (data-parallel over query rows, keys replicated):
  - core c handles query rows [c*2048, (c+1)*2048)
  - device computes, per query, the top-24 candidate neighbor *indices* using
    an augmented fp32 matmul score  s(q,j) = 2*x_q.x_j - |x_j|^2  (monotone
    decreasing in d2) and the DVE top-8 primitives (max / max_index /
    match_replace)
  - host rescores the (at most) 24 candidates per row in fp64, orders them with
    exact reference tie semantics (score asc, index asc) and emits edges +
    logprobs.  Host work is O(N*24*D), i.e. ~0.15% of the device FLOPs.
"""

import sys

for _p in ("/opt/trn_rl_repo",):
    if _p not in sys.path:
        sys.path.insert(0, _p)

import numpy as np

import concourse.bass as bass
import concourse.bacc as bacc
import concourse.mybir as mybir
from concourse.bass_utils import run_bass_kernel_spmd
from concourse.masks import make_identity
from concourse.tile import TileContext

N = 16384
D = 64
K = 20
NCORES = 8
QPC = N // NCORES       # queries per core
P = 128                 # query tile rows (SBUF partitions)
NQT = QPC // P          # query tiles per core
SL = 512                # stage-1 slice width (keys per chunk)
NSL = N // SL           # 32 chunks
CAND = NSL * 8          # 256 candidates per row after stage 1
ROUNDS = 3              # 8 extracted per round -> 24 candidates out
NEXT = ROUNDS * 8
NEG = -1.0e30

_F32 = mybir.dt.float32
_U32 = mybir.dt.uint32


def _build_nc() -> bass.Bass:
    nc = bacc.Bacc()
    x = nc.declare_dram_parameter("x", [N, D], _F32, isOutput=False)
    xq = nc.declare_dram_parameter("xq", [QPC, D], _F32, isOutput=False)
    idx_out = nc.declare_dram_parameter("idx_out", [QPC, NEXT], _F32, isOutput=True)
    val_out = nc.declare_dram_parameter("val_out", [QPC, NEXT], _F32, isOutput=True)

    with TileContext(nc) as tc:
        with (
            tc.tile_pool(name="const", bufs=1) as constp,
            tc.tile_pool(name="load", bufs=4) as loadp,
            tc.tile_pool(name="pst", bufs=2, space="PSUM") as pstp,
            tc.tile_pool(name="big", bufs=1) as bigp,
            tc.tile_pool(name="chunk", bufs=6) as chunkp,
            tc.tile_pool(name="psd", bufs=4, space="PSUM") as psdp,
            tc.tile_pool(name="cand", bufs=2) as candp,
            tc.tile_pool(name="outp", bufs=2) as outp,
        ):
            ident = constp.tile([P, P], _F32)
            make_identity(nc, ident)
            ones64 = constp.tile([D, 1], _F32)
            nc.vector.memset(ones64, 1.0)
            negbig = constp.tile([P, CAND], _F32)
            nc.vector.memset(negbig, NEG)
            # base_idx[p, s*8 + r] = s*SL
            base_idx = constp.tile([P, CAND], _U32)
            nc.gpsimd.iota(
                base_idx, pattern=[[SL, NSL], [0, 8]], base=0, channel_multiplier=0
            )

            # keys, feature-major, with augmentation row 64 = -|x_j|^2
            xkaug = bigp.tile([D + 1, N], _F32)
            for c in range(N // P):
                xt = loadp.tile([P, D], _F32, tag="xt")
                nc.sync.dma_start(out=xt, in_=x[c * P : (c + 1) * P, :])
                ps = pstp.tile([D, P], _F32, tag="pst_t")
                nc.tensor.transpose(ps, xt, ident)
                nc.scalar.copy(xkaug[0:D, c * P : (c + 1) * P], ps)
            for ch in range(NSL):
                sl = slice(ch * SL, (ch + 1) * SL)
                xsq = loadp.tile([D, SL], _F32, tag="xsq")
                nc.vector.tensor_mul(xsq, xkaug[0:D, sl], xkaug[0:D, sl])
                ps1 = pstp.tile([1, SL], _F32, tag="pst_s")
                nc.tensor.matmul(ps1, ones64, xsq, start=True, stop=True)
                nc.scalar.mul(xkaug[D : D + 1, sl], ps1, -1.0)

            # queries, feature-major, scaled by 2, augmentation row 64 = 1
            xqaug = bigp.tile([D + 1, QPC], _F32)
            nc.vector.memset(xqaug[D : D + 1, :], 1.0)
            for t in range(NQT):
                xtq = loadp.tile([P, D], _F32, tag="xtq")
                nc.sync.dma_start(out=xtq, in_=xq[t * P : (t + 1) * P, :])
                psq = pstp.tile([D, P], _F32, tag="pst_t")
                nc.tensor.transpose(psq, xtq, ident)
                nc.scalar.mul(xqaug[0:D, t * P : (t + 1) * P], psq, 2.0)

            for t in range(NQT):
                lhs = xqaug[:, t * P : (t + 1) * P]
                cand = candp.tile([P, CAND], _F32, tag="cand")
                candloc = candp.tile([P, CAND], _U32, tag="candloc")
                for ch in range(NSL):
                    s8 = slice(ch * 8, (ch + 1) * 8)
                    psd = psdp.tile([P, SL], _F32, tag="psd")
                    nc.tensor.matmul(
                        psd,
                        lhs,
                        xkaug[:, ch * SL : (ch + 1) * SL],
                        start=True,
                        stop=True,
                    )
                    dchunk = chunkp.tile([P, SL], _F32, tag="dchunk")
                    nc.scalar.copy(dchunk, psd)
                    nc.vector.max(out=cand[:, s8], in_=dchunk)
                    nc.vector.max_index(
                        out=candloc[:, s8], in_max=cand[:, s8], in_values=dchunk
                    )

                # global candidate indices, as exact fp32 values
                candg = candp.tile([P, CAND], _U32, tag="candg")
                nc.vector.tensor_add(candg, candloc, base_idx)
                candgf = candp.tile([P, CAND], _F32, tag="candgf")
                nc.vector.tensor_copy(candgf, candg)

                # mark top-NEXT values in cand (destructive)
                gv = outp.tile([P, NEXT], _F32, tag="gv")
                for r in range(ROUNDS):
                    r8 = slice(r * 8, (r + 1) * 8)
                    nc.vector.max(out=gv[:, r8], in_=cand)
                    nc.vector.match_replace(
                        out=cand, in_to_replace=gv[:, r8], in_values=cand, imm_value=NEG
                    )
                mask = candp.tile([P, CAND], _U32, tag="mask")
                nc.vector.tensor_scalar(
                    mask, cand, NEG, None, op0=mybir.AluOpType.is_equal
                )
                idxm = candp.tile([P, CAND], _F32, tag="idxm")
                nc.vector.select(idxm, mask, candgf, negbig)

                # extract the marked candidates' global indices
                hv = outp.tile([P, NEXT], _F32, tag="hv")
                for r in range(ROUNDS):
                    r8 = slice(r * 8, (r + 1) * 8)
                    nc.vector.max(out=hv[:, r8], in_=idxm)
                    nc.vector.match_replace(
                        out=idxm, in_to_replace=hv[:, r8], in_values=idxm, imm_value=NEG
                    )

                nc.sync.dma_start(out=idx_out[t * P : (t + 1) * P, :], in_=hv)
                nc.sync.dma_start(out=val_out[t * P : (t + 1) * P, :], in_=gv)

    nc.compile()
    return nc


_NC_CACHE = None


def _get_nc() -> bass.Bass:
    global _NC_CACHE
    if _NC_CACHE is None:
        _NC_CACHE = _build_nc()
    return _NC_CACHE


def run_device(x: np.ndarray, trace: bool = False):
    """Run the bass kernel on all 8 cores. Returns (idx [N, NEXT] int64,
    BassKernelResults)."""
    x = np.ascontiguousarray(x, dtype=np.float32)
    nc = _get_nc()
    in_maps = [
        {"x": x, "xq": np.ascontiguousarray(x[c * QPC : (c + 1) * QPC])}
        for c in range(NCORES)
    ]
    res = run_bass_kernel_spmd(nc, in_maps, list(range(NCORES)), trace=trace)
    idx_f = np.concatenate([r["idx_out"] for r in res.results], axis=0)
    idx = idx_f.astype(np.int64)
    if (idx_f < 0).any() or (idx_f > N - 1).any():
        raise RuntimeError("device returned out-of-range candidate index")
    return idx, res


def _finish_host(x: np.ndarray, temperature: np.ndarray, idx: np.ndarray):
    """fp64 rescore of device candidates + exact reference ordering."""
    x64 = x.astype(np.float64)
    xg = x64[idx]                                    # [N, NEXT, D]
    d2 = np.square(xg - x64[:, None, :]).sum(-1)     # [N, NEXT] fp64
    d32 = d2.astype(np.float32)
    t32 = np.float32(np.clip(np.float32(temperature), -5.0, 5.0))
    scale = np.float32(np.exp(t32))
    key = -(d32 * scale)                             # fp32, like the reference
    # top-K largest of key; ties -> lower index first (top_k stability)
    order = np.lexsort((idx, -key.astype(np.float64)), axis=-1)[:, :K]
    top_idx = np.take_along_axis(idx, order, axis=1)
    top_lp = np.take_along_axis(key, order, axis=1)
    edges = np.stack(
        [
            top_idx.reshape(-1).astype(np.int32),
            np.repeat(np.arange(N, dtype=np.int32), K),
        ],
        axis=0,
    )
    logprobs = top_lp.reshape(1, N, K).astype(np.float32)
    return edges, logprobs


def kernel(x: np.ndarray, A: np.ndarray, temperature: np.ndarray):
    x = np.ascontiguousarray(x, dtype=np.float32)
    idx, _ = run_device(x)
    edges, logprobs = _finish_host(x, np.asarray(temperature), idx)
    return (x[None], edges, logprobs)


if __name__ == "__main__":
    rng = np.random.default_rng(0)
    x = rng.standard_normal((N, D), dtype=np.float32)
    out = kernel(x, np.zeros((1,), np.int32), np.float32(4.0))
    print([o.shape for o in out], [o.dtype for o in out])
